# revision 11
# baseline (speedup 1.0000x reference)
"""Trainium2 Bass kernel: BayesPredictor kNN-softmax (retrieval_knn).

Math (reference):
    a_b   = cosine alpha-bar schedule(ts_b), clipped
    v_b   = 1 - a_b
    z[b,n] = -(0.5*D*log v_b + 0.5/v_b * ||x_b - sqrt(a_b) y_n||^2)
    probs  = softmax_n(z);  x0 = probs @ Y;  eps = (x - sqrt(a) x0)/sqrt(v)

Per-row-constant terms of z drop out of the softmax, so each core only needs
    z'[b,n] = g_b * <x_b, y_n> - h_b * ||y_n||^2,
    g_b = sqrt(a_b)/v_b,  h_b = 0.5*a_b/v_b.

Sharding: data_batch split over N across 8 cores (flash-style sharded
softmax).  Each core computes z' for its N/8 rows via one matmul whose
stationary operand folds the per-b coefficients:
    lhsT column b = [g_b * x_b ; h_b ; 0-pad]   ([D+128, B] on host)
    rhs row block = [y_n^T    ; -||y_n||^2 ; 0-pad]
then row-max m, e = exp(z' - m), l = sum e, S = e @ Y_loc (second matmul,
with e transposed on-chip via the PE).  The host merges the 8 partial
(m, l, S) triples with a log-sum-exp merge and finishes eps.
"""

import numpy as np

import concourse.tile as tile
from concourse import bacc, mybir
from concourse.bass_utils import run_bass_kernel_spmd
from concourse.masks import make_identity

P = 128
F32 = mybir.dt.float32
AX = mybir.AxisListType.X
EXP = mybir.ActivationFunctionType.Exp

NUM_TIMESTEPS = 1000


def build_module(B=256, D=3072, NLOC=2048, num_devices=8, use_f32r=True):
    """Build the per-core Bass module (same program on every core).

    use_f32r: run the two big matmuls in float32r (TF32-class single-pass PE
    mode, 4x the fp32 rate; measured maxerr ~0.03 on a D=3072 N(0,1)
    contraction vs 0.57 for bf16).  The ||y||^2 fold row stays fp32: its
    ~3072 magnitude would turn the truncation into a distance error larger
    than typical top-2 gaps.  The BIR verifier requires every producer
    feeding an FP32r matmul to emit FP32r, so the main-path dram/sbuf
    tensors are declared float32r (same 4-byte storage).
    """
    assert B % P == 0 and D % P == 0 and NLOC % P == 0
    MMDT = mybir.dt.float32r if use_f32r else F32

    BT = B // P                    # output-row tiles
    KM = D // P                    # mm1 main contraction tiles
    NCH_SZ = min(512, NLOC)        # mm1 psum chunk (free dim)
    NCH = NLOC // NCH_SZ           # chunks per b-tile
    assert BT * NCH <= 8, "mm1 needs all live psum chunks <= 8 banks"
    KT2 = NLOC // P                # mm2 contraction tiles
    DCH_SZ = min(512, D)           # mm2 psum chunk
    DCH = D // DCH_SZ              # total output chunks per b-tile
    gmax = min(DCH, max(1, 8 // BT))       # psum budget per mm2 pass
    groups = []                            # chunk counts per pass, small tail last
    left = DCH
    while left > 0:
        g = min(gmax, left)
        groups.append(g)
        left -= g
    groups.sort(reverse=True)

    nc = bacc.Bacc(
        "TRN2",
        target_bir_lowering=False,
        debug=False,
        enable_asserts=False,
        num_devices=num_devices,
    )

    xt = nc.dram_tensor("xt", [KM, P, B], MMDT, kind="ExternalInput").ap()
    xtf = nc.dram_tensor("xtf", [1, B], F32, kind="ExternalInput").ap()
    ytk = nc.dram_tensor("ytk", [KM, P, NLOC], MMDT, kind="ExternalInput").ap()
    ytkf = nc.dram_tensor("ytkf", [1, NLOC], F32, kind="ExternalInput").ap()
    yn = nc.dram_tensor("yn", [NLOC, D], MMDT, kind="ExternalInput").ap()
    s_out = nc.dram_tensor("s", [BT, P, D], F32, kind="ExternalOutput").ap()
    mneg_out = nc.dram_tensor("mneg", [BT, P, 1], F32, kind="ExternalOutput").ap()
    l_out = nc.dram_tensor("l", [BT, P, 1], F32, kind="ExternalOutput").ap()

    with tile.TileContext(nc) as tc:
        with (
            tc.tile_pool(name="const", bufs=1) as const,
            tc.tile_pool(name="dt1", bufs=8) as dt1,
            tc.tile_pool(name="dtf", bufs=1) as dtf_pool,
            tc.tile_pool(name="epool", bufs=2) as epool,
            tc.tile_pool(name="etpool", bufs=2) as etpool,
            tc.tile_pool(name="stat", bufs=4) as stat,
            tc.tile_pool(name="dt2", bufs=8) as dt2,
            tc.tile_pool(name="opool", bufs=4) as opool,
            tc.tile_pool(name="psum", bufs=8, space="PSUM") as psum,
        ):
            # constants: folded/scaled x^T (all k tiles) + identity for transpose
            xt_sb = const.tile([P, KM, B], MMDT)
            for k in range(KM):
                nc.sync.dma_start(xt_sb[:, k, :], xt[k])
            xtf_sb = const.tile([P, B], F32)
            nc.vector.memset(xtf_sb[:], 0.0)
            nc.sync.dma_start(xtf_sb[0:1, :], xtf[:])
            ident = const.tile([P, P], F32)
            make_identity(nc, ident)

            # ---- mm1: z'[bt][c] += xt_k.T @ ytk_k  (+ fp32 fold tile last)
            zt = [
                [psum.tile([P, NCH_SZ], F32, tag="ps", name=f"z_{bt}_{c}") for c in range(NCH)]
                for bt in range(BT)
            ]
            for k in range(KM + 1):
                fold = k == KM
                if fold:
                    dtile = dtf_pool.tile([P, NLOC], F32, tag="dtf", name="dtile_f")
                    nc.vector.memset(dtile[:], 0.0)
                    nc.sync.dma_start(dtile[0:1, :], ytkf[:])
                else:
                    dtile = dt1.tile([P, NLOC], MMDT, tag="dt", name="dtile")
                    nc.sync.dma_start(dtile[:], ytk[k])
                for bt in range(BT):
                    lhsT = (xtf_sb if fold else xt_sb[:, k])[:, bt * P : (bt + 1) * P]
                    for c in range(NCH):
                        nc.tensor.matmul(
                            zt[bt][c][:],
                            lhsT,
                            dtile[:, c * NCH_SZ : (c + 1) * NCH_SZ],
                            start=(k == 0),
                            stop=fold,
                        )

            # ---- row stats + exp + on-chip transpose of e
            et_tiles = []
            for bt in range(BT):
                mtmp = stat.tile([P, NCH], F32, tag="mtmp")
                for c in range(NCH):
                    nc.vector.reduce_max(mtmp[:, c : c + 1], zt[bt][c][:], axis=AX)
                negm = stat.tile([P, 1], F32, tag="negm")
                nc.vector.reduce_max(negm[:], mtmp[:], axis=AX, negate=True)

                e_sb = epool.tile([P, NLOC], F32, tag="e")
                lpart = stat.tile([P, NCH], F32, tag="lpart")
                for c in range(NCH):
                    nc.scalar.activation(
                        e_sb[:, c * NCH_SZ : (c + 1) * NCH_SZ],
                        zt[bt][c][:],
                        EXP,
                        bias=negm[:],
                        accum_out=lpart[:, c : c + 1],
                    )
                lsum = stat.tile([P, 1], F32, tag="lsum")
                nc.vector.reduce_sum(lsum[:], lpart[:], axis=AX)
                nc.sync.dma_start(mneg_out[bt], negm[:])
                nc.sync.dma_start(l_out[bt], lsum[:])

                et_sb = etpool.tile([P, KT2, P], MMDT, tag="et")
                for kk in range(KT2):
                    ps = psum.tile([P, P], F32, tag="ps")
                    nc.tensor.transpose(
                        ps[:], e_sb[:, kk * P : (kk + 1) * P], ident[:]
                    )
                    nc.any.tensor_copy(out=et_sb[:, kk, :], in_=ps[:])
                et_tiles.append(et_sb)

            # ---- mm2: S[bt] = e[bt] @ Y_loc, done in `passes` column groups
            gstart = [sum(groups[:i]) for i in range(len(groups))]
            for g, group in enumerate(groups):
                d0 = gstart[g] * DCH_SZ
                dw = group * DCH_SZ
                xo = [
                    [psum.tile([P, DCH_SZ], F32, tag="ps", name=f"xo_{g}_{bt}_{c}") for c in range(group)]
                    for bt in range(BT)
                ]
                for k in range(KT2):
                    ytile = dt2.tile([P, dw], MMDT, tag="d2")
                    nc.sync.dma_start(
                        ytile[:], yn[k * P : (k + 1) * P, d0 : d0 + dw]
                    )
                    for bt in range(BT):
                        lhsT = et_tiles[bt][:, k, :]
                        for c in range(group):
                            nc.tensor.matmul(
                                xo[bt][c][:],
                                lhsT,
                                ytile[:, c * DCH_SZ : (c + 1) * DCH_SZ],
                                start=(k == 0),
                                stop=(k == KT2 - 1),
                            )
                for bt in range(BT):
                    for c in range(group):
                        o = opool.tile([P, DCH_SZ], F32, tag="o")
                        nc.any.tensor_copy(out=o[:], in_=xo[bt][c][:])
                        nc.sync.dma_start(
                            s_out[bt][:, d0 + c * DCH_SZ : d0 + (c + 1) * DCH_SZ],
                            o[:],
                        )

    nc.compile()
    return nc


def host_prep(inputs, ts, data_batch, n_cores=8):
    """Schedule coefficients + folded operands, all in the reference's f32."""
    x = np.asarray(inputs, dtype=np.float32)
    y = np.asarray(data_batch, dtype=np.float32)
    B, D = x.shape
    N = y.shape[0]
    NLOC = N // n_cores

    # cosine alpha-bar schedule, matching reference f32 arithmetic
    s = np.float32(0.008)
    tsf = np.asarray(ts).astype(np.float32)
    f = np.cos(((tsf / np.float32(NUM_TIMESTEPS)) + s) / (np.float32(1.0) + s)
               * np.float32(np.pi / 2)) ** 2
    f0 = np.cos(np.float32(s / (1.0 + s) * (np.pi / 2))) ** 2
    alphas = np.clip(f.astype(np.float32) / f0.astype(np.float32), 1e-5, 1.0 - 1e-5)
    alphas = alphas.astype(np.float64)
    v = 1.0 - alphas
    sqrt_a = np.sqrt(alphas)
    g = sqrt_a / v
    h = 0.5 * alphas / v

    # folded stationary operand: [g_b * x_b] tiles + separate [h_b] fold row
    xt_aug = (x.astype(np.float64) * g[:, None]).T.astype(np.float32)
    xt_tiled = np.ascontiguousarray(xt_aug.reshape(D // P, P, B))
    xtf = h.astype(np.float32).reshape(1, B)

    ysq = np.sum(y.astype(np.float64) ** 2, axis=1)

    in_maps = []
    for c in range(n_cores):
        yc = y[c * NLOC : (c + 1) * NLOC]
        ytk = np.ascontiguousarray(yc.T.reshape(D // P, P, NLOC))
        ytkf = (-ysq[c * NLOC : (c + 1) * NLOC]).astype(np.float32).reshape(1, NLOC)
        in_maps.append(
            {
                "xt": xt_tiled,
                "xtf": xtf,
                "ytk": ytk,
                "ytkf": ytkf,
                "yn": np.ascontiguousarray(yc),
            }
        )
    return in_maps, alphas, v, sqrt_a


def host_merge(results, x, alphas, v, sqrt_a):
    """Log-sum-exp merge of the per-core partial softmax results."""
    B, D = x.shape
    m_i = np.stack([-r["mneg"].reshape(B).astype(np.float64) for r in results])
    l_i = np.stack([r["l"].reshape(B).astype(np.float64) for r in results])
    s_i = np.stack([r["s"].reshape(B, D).astype(np.float64) for r in results])

    m = m_i.max(axis=0)
    w = np.exp(m_i - m)                        # [n_cores, B]
    l = (w * l_i).sum(axis=0)                  # [B]
    s = (w[:, :, None] * s_i).sum(axis=0)      # [B, D]
    x0 = s / l[:, None]
    eps = (x.astype(np.float64) - sqrt_a[:, None] * x0) / np.sqrt(v)[:, None]
    return eps.astype(np.float32)


_CACHE = {}


def _get_module():
    if "nc" not in _CACHE:
        _CACHE["nc"] = build_module()
    return _CACHE["nc"]


def kernel(inputs, ts, data_batch, _trace=False):
    n_cores = 8
    x = np.asarray(inputs, dtype=np.float32)
    in_maps, alphas, v, sqrt_a = host_prep(inputs, ts, data_batch, n_cores)
    nc = _get_module()
    res = run_bass_kernel_spmd(nc, in_maps, list(range(n_cores)), trace=_trace)
    out = host_merge(res.results, x, alphas, v, sqrt_a)
    if _trace:
        _CACHE["last_results"] = res
    return out


# revision 12
# speedup vs baseline: 1.1849x; 1.1849x over previous
"""Trainium2 Bass kernel: BayesPredictor kNN-softmax (retrieval_knn).

Math (reference):
    a_b   = cosine alpha-bar schedule(ts_b), clipped
    v_b   = 1 - a_b
    z[b,n] = -(0.5*D*log v_b + 0.5/v_b * ||x_b - sqrt(a_b) y_n||^2)
    probs  = softmax_n(z);  x0 = probs @ Y;  eps = (x - sqrt(a) x0)/sqrt(v)

Per-row-constant terms of z drop out of the softmax, so each core only needs
    z'[b,n] = g_b * <x_b, y_n> - h_b * ||y_n||^2,
    g_b = sqrt(a_b)/v_b,  h_b = 0.5*a_b/v_b.

Sharding: data_batch split over N across 8 cores (flash-style sharded
softmax).  Each core computes z' for its N/8 rows via one matmul whose
stationary operand folds the per-b coefficients:
    lhsT column b = [g_b * x_b ; h_b ; 0-pad]   ([D+128, B] on host)
    rhs row block = [y_n^T    ; -||y_n||^2 ; 0-pad]
then row-max m, e = exp(z' - m), l = sum e, S = e @ Y_loc (second matmul,
with e transposed on-chip via the PE).  The host merges the 8 partial
(m, l, S) triples with a log-sum-exp merge and finishes eps.
"""

import numpy as np

import concourse.tile as tile
from concourse import bacc, mybir
from concourse.bass_utils import run_bass_kernel_spmd
from concourse.masks import make_identity

P = 128
F32 = mybir.dt.float32
AX = mybir.AxisListType.X
EXP = mybir.ActivationFunctionType.Exp

NUM_TIMESTEPS = 1000


def build_module(B=256, D=3072, NLOC=2048, num_devices=8, use_f32r=True):
    """Build the per-core Bass module (same program on every core).

    use_f32r: run the two big matmuls in float32r (TF32-class single-pass PE
    mode, 4x the fp32 rate; measured maxerr ~0.03 on a D=3072 N(0,1)
    contraction vs 0.57 for bf16).  The ||y||^2 fold row stays fp32: its
    ~3072 magnitude would turn the truncation into a distance error larger
    than typical top-2 gaps.  The BIR verifier requires every producer
    feeding an FP32r matmul to emit FP32r, so the main-path dram/sbuf
    tensors are declared float32r (same 4-byte storage).
    """
    assert B % P == 0 and D % P == 0 and NLOC % P == 0
    MMDT = mybir.dt.float32r if use_f32r else F32

    BT = B // P                    # output-row tiles
    KM = D // P                    # mm1 main contraction tiles
    NCH_SZ = min(512, NLOC)        # mm1 psum chunk (free dim)
    NCH = NLOC // NCH_SZ           # chunks per b-tile
    assert BT * NCH <= 8, "mm1 needs all live psum chunks <= 8 banks"
    KT2 = NLOC // P                # mm2 contraction tiles
    DCH_SZ = min(512, D)           # mm2 psum chunk
    DCH = D // DCH_SZ              # total output chunks per b-tile
    group = min(DCH, max(1, 8 // BT - 1))  # chunks per mm2 pass (psum budget)
    while DCH % group:
        group -= 1
    groups = [group] * (DCH // group)

    nc = bacc.Bacc(
        "TRN2",
        target_bir_lowering=False,
        debug=False,
        enable_asserts=False,
        num_devices=num_devices,
    )

    xt = nc.dram_tensor("xt", [KM, P, B], MMDT, kind="ExternalInput").ap()
    xtf = nc.dram_tensor("xtf", [1, B], F32, kind="ExternalInput").ap()
    ytk = nc.dram_tensor("ytk", [KM, P, NLOC], MMDT, kind="ExternalInput").ap()
    ytkf = nc.dram_tensor("ytkf", [1, NLOC], F32, kind="ExternalInput").ap()
    yn = nc.dram_tensor("yn", [NLOC, D], MMDT, kind="ExternalInput").ap()
    s_out = nc.dram_tensor("s", [BT, P, D], F32, kind="ExternalOutput").ap()
    mneg_out = nc.dram_tensor("mneg", [BT, P, 1], F32, kind="ExternalOutput").ap()
    l_out = nc.dram_tensor("l", [BT, P, 1], F32, kind="ExternalOutput").ap()

    with tile.TileContext(nc) as tc:
        with (
            tc.tile_pool(name="const", bufs=1) as const,
            tc.tile_pool(name="dt1", bufs=8) as dt1,
            tc.tile_pool(name="dtf", bufs=1) as dtf_pool,
            tc.tile_pool(name="epool", bufs=2) as epool,
            tc.tile_pool(name="etpool", bufs=2) as etpool,
            tc.tile_pool(name="stat", bufs=4) as stat,
            tc.tile_pool(name="dt2", bufs=8) as dt2,
            tc.tile_pool(name="opool", bufs=4) as opool,
            tc.tile_pool(name="psum", bufs=8, space="PSUM") as psum,
        ):
            # constants: folded/scaled x^T (all k tiles) + identity for transpose
            xt_sb = const.tile([P, KM, B], MMDT)
            for k in range(KM):
                nc.sync.dma_start(xt_sb[:, k, :], xt[k])
            xtf_sb = const.tile([P, B], F32)
            nc.vector.memset(xtf_sb[:], 0.0)
            nc.sync.dma_start(xtf_sb[0:1, :], xtf[:])
            ident = const.tile([P, P], F32)
            make_identity(nc, ident)

            # ---- mm1: z'[bt][c] += xt_k.T @ ytk_k  (+ fp32 fold tile last)
            zt = [
                [psum.tile([P, NCH_SZ], F32, tag="ps", name=f"z_{bt}_{c}") for c in range(NCH)]
                for bt in range(BT)
            ]
            for k in range(KM + 1):
                fold = k == KM
                if fold:
                    dtile = dtf_pool.tile([P, NLOC], F32, tag="dtf", name="dtile_f")
                    nc.vector.memset(dtile[:], 0.0)
                    nc.sync.dma_start(dtile[0:1, :], ytkf[:])
                else:
                    dtile = dt1.tile([P, NLOC], MMDT, tag="dt", name="dtile")
                    nc.sync.dma_start(dtile[:], ytk[k])
                for bt in range(BT):
                    lhsT = (xtf_sb if fold else xt_sb[:, k])[:, bt * P : (bt + 1) * P]
                    for c in range(NCH):
                        nc.tensor.matmul(
                            zt[bt][c][:],
                            lhsT,
                            dtile[:, c * NCH_SZ : (c + 1) * NCH_SZ],
                            start=(k == 0),
                            stop=fold,
                        )

            # ---- row stats + exp + on-chip transpose of e
            et_tiles = []
            for bt in range(BT):
                mtmp = stat.tile([P, NCH], F32, tag="mtmp")
                for c in range(NCH):
                    nc.vector.reduce_max(mtmp[:, c : c + 1], zt[bt][c][:], axis=AX)
                negm = stat.tile([P, 1], F32, tag="negm")
                nc.vector.reduce_max(negm[:], mtmp[:], axis=AX, negate=True)

                e_sb = epool.tile([P, NLOC], F32, tag="e")
                lpart = stat.tile([P, NCH], F32, tag="lpart")
                for c in range(NCH):
                    nc.scalar.activation(
                        e_sb[:, c * NCH_SZ : (c + 1) * NCH_SZ],
                        zt[bt][c][:],
                        EXP,
                        bias=negm[:],
                        accum_out=lpart[:, c : c + 1],
                    )
                lsum = stat.tile([P, 1], F32, tag="lsum")
                nc.vector.reduce_sum(lsum[:], lpart[:], axis=AX)
                nc.sync.dma_start(mneg_out[bt], negm[:])
                nc.sync.dma_start(l_out[bt], lsum[:])

                et_sb = etpool.tile([P, KT2, P], MMDT, tag="et")
                for kk in range(KT2):
                    ps = psum.tile([P, P], F32, tag="ps")
                    nc.tensor.transpose(
                        ps[:], e_sb[:, kk * P : (kk + 1) * P], ident[:]
                    )
                    nc.any.tensor_copy(out=et_sb[:, kk, :], in_=ps[:])
                et_tiles.append(et_sb)

            # ---- mm2: S[bt] = e[bt] @ Y_loc, done in `passes` column groups
            gstart = [sum(groups[:i]) for i in range(len(groups))]
            for g, group in enumerate(groups):
                d0 = gstart[g] * DCH_SZ
                dw = group * DCH_SZ
                xo = [
                    [psum.tile([P, DCH_SZ], F32, tag="ps", name=f"xo_{g}_{bt}_{c}") for c in range(group)]
                    for bt in range(BT)
                ]
                for k in range(KT2):
                    ytile = dt2.tile([P, dw], MMDT, tag="d2")
                    nc.sync.dma_start(
                        ytile[:], yn[k * P : (k + 1) * P, d0 : d0 + dw]
                    )
                    for bt in range(BT):
                        lhsT = et_tiles[bt][:, k, :]
                        for c in range(group):
                            nc.tensor.matmul(
                                xo[bt][c][:],
                                lhsT,
                                ytile[:, c * DCH_SZ : (c + 1) * DCH_SZ],
                                start=(k == 0),
                                stop=(k == KT2 - 1),
                            )
                for bt in range(BT):
                    for c in range(group):
                        o = opool.tile([P, DCH_SZ], F32, tag="o")
                        nc.any.tensor_copy(out=o[:], in_=xo[bt][c][:])
                        nc.sync.dma_start(
                            s_out[bt][:, d0 + c * DCH_SZ : d0 + (c + 1) * DCH_SZ],
                            o[:],
                        )

    nc.compile()
    return nc


def host_prep(inputs, ts, data_batch, n_cores=8):
    """Schedule coefficients + folded operands, all in the reference's f32."""
    x = np.asarray(inputs, dtype=np.float32)
    y = np.asarray(data_batch, dtype=np.float32)
    B, D = x.shape
    N = y.shape[0]
    NLOC = N // n_cores

    # cosine alpha-bar schedule, matching reference f32 arithmetic
    s = np.float32(0.008)
    tsf = np.asarray(ts).astype(np.float32)
    f = np.cos(((tsf / np.float32(NUM_TIMESTEPS)) + s) / (np.float32(1.0) + s)
               * np.float32(np.pi / 2)) ** 2
    f0 = np.cos(np.float32(s / (1.0 + s) * (np.pi / 2))) ** 2
    alphas = np.clip(f.astype(np.float32) / f0.astype(np.float32), 1e-5, 1.0 - 1e-5)
    alphas = alphas.astype(np.float64)
    v = 1.0 - alphas
    sqrt_a = np.sqrt(alphas)
    g = sqrt_a / v
    h = 0.5 * alphas / v

    # folded stationary operand: [g_b * x_b] tiles + separate [h_b] fold row
    xt_aug = (x.astype(np.float64) * g[:, None]).T.astype(np.float32)
    xt_tiled = np.ascontiguousarray(xt_aug.reshape(D // P, P, B))
    xtf = h.astype(np.float32).reshape(1, B)

    ysq = np.sum(y.astype(np.float64) ** 2, axis=1)

    in_maps = []
    for c in range(n_cores):
        yc = y[c * NLOC : (c + 1) * NLOC]
        ytk = np.ascontiguousarray(yc.T.reshape(D // P, P, NLOC))
        ytkf = (-ysq[c * NLOC : (c + 1) * NLOC]).astype(np.float32).reshape(1, NLOC)
        in_maps.append(
            {
                "xt": xt_tiled,
                "xtf": xtf,
                "ytk": ytk,
                "ytkf": ytkf,
                "yn": np.ascontiguousarray(yc),
            }
        )
    return in_maps, alphas, v, sqrt_a


def host_merge(results, x, alphas, v, sqrt_a):
    """Log-sum-exp merge of the per-core partial softmax results."""
    B, D = x.shape
    m_i = np.stack([-r["mneg"].reshape(B).astype(np.float64) for r in results])
    l_i = np.stack([r["l"].reshape(B).astype(np.float64) for r in results])
    s_i = np.stack([r["s"].reshape(B, D).astype(np.float64) for r in results])

    m = m_i.max(axis=0)
    w = np.exp(m_i - m)                        # [n_cores, B]
    l = (w * l_i).sum(axis=0)                  # [B]
    s = (w[:, :, None] * s_i).sum(axis=0)      # [B, D]
    x0 = s / l[:, None]
    eps = (x.astype(np.float64) - sqrt_a[:, None] * x0) / np.sqrt(v)[:, None]
    return eps.astype(np.float32)


_CACHE = {}


def _get_module():
    if "nc" not in _CACHE:
        _CACHE["nc"] = build_module()
    return _CACHE["nc"]


def kernel(inputs, ts, data_batch, _trace=False):
    n_cores = 8
    x = np.asarray(inputs, dtype=np.float32)
    in_maps, alphas, v, sqrt_a = host_prep(inputs, ts, data_batch, n_cores)
    nc = _get_module()
    res = run_bass_kernel_spmd(nc, in_maps, list(range(n_cores)), trace=_trace)
    out = host_merge(res.results, x, alphas, v, sqrt_a)
    if _trace:
        _CACHE["last_results"] = res
    return out


# revision 14
# speedup vs baseline: 1.2058x; 1.0176x over previous
"""Trainium2 Bass kernel: BayesPredictor kNN-softmax (retrieval_knn).

Math (reference):
    a_b   = cosine alpha-bar schedule(ts_b), clipped
    v_b   = 1 - a_b
    z[b,n] = -(0.5*D*log v_b + 0.5/v_b * ||x_b - sqrt(a_b) y_n||^2)
    probs  = softmax_n(z);  x0 = probs @ Y;  eps = (x - sqrt(a) x0)/sqrt(v)

Per-row-constant terms of z drop out of the softmax, so each core only needs
    z'[b,n] = g_b * <x_b, y_n> - h_b * ||y_n||^2,
    g_b = sqrt(a_b)/v_b,  h_b = 0.5*a_b/v_b.

Sharding: data_batch split over N across 8 cores (flash-style sharded
softmax).  Each core computes z' for its N/8 rows via one matmul whose
stationary operand folds the per-b coefficients:
    lhsT column b = [g_b * x_b ; h_b ; 0-pad]   ([D+128, B] on host)
    rhs row block = [y_n^T    ; -||y_n||^2 ; 0-pad]
then row-max m, e = exp(z' - m), l = sum e, S = e @ Y_loc (second matmul,
with e transposed on-chip via the PE).  The host merges the 8 partial
(m, l, S) triples with a log-sum-exp merge and finishes eps.
"""

import numpy as np

import concourse.tile as tile
from concourse import bacc, mybir
from concourse.bass_utils import run_bass_kernel_spmd
from concourse.masks import make_identity

P = 128
F32 = mybir.dt.float32
AX = mybir.AxisListType.X
EXP = mybir.ActivationFunctionType.Exp

NUM_TIMESTEPS = 1000


def build_module(B=256, D=3072, NLOC=2048, num_devices=8, use_f32r=True):
    """Build the per-core Bass module (same program on every core).

    use_f32r: run the two big matmuls in float32r (TF32-class single-pass PE
    mode, 4x the fp32 rate; measured maxerr ~0.03 on a D=3072 N(0,1)
    contraction vs 0.57 for bf16).  The ||y||^2 fold row stays fp32: its
    ~3072 magnitude would turn the truncation into a distance error larger
    than typical top-2 gaps.  The BIR verifier requires every producer
    feeding an FP32r matmul to emit FP32r, so the main-path dram/sbuf
    tensors are declared float32r (same 4-byte storage).
    """
    assert B % P == 0 and D % P == 0 and NLOC % P == 0
    MMDT = mybir.dt.float32r if use_f32r else F32

    BT = B // P                    # output-row tiles
    KM = D // P                    # mm1 main contraction tiles
    NCH_SZ = min(512, NLOC)        # mm1 psum chunk (free dim)
    NCH = NLOC // NCH_SZ           # chunks per b-tile
    assert BT * NCH <= 8, "mm1 needs all live psum chunks <= 8 banks"
    KT2 = NLOC // P                # mm2 contraction tiles
    DCH_SZ = min(512, D)           # mm2 psum chunk
    DCH = D // DCH_SZ              # total output chunks per b-tile
    group = min(DCH, max(1, 8 // BT - 1))  # chunks per mm2 pass (psum budget)
    while DCH % group:
        group -= 1
    groups = [group] * (DCH // group)

    nc = bacc.Bacc(
        "TRN2",
        target_bir_lowering=False,
        debug=False,
        enable_asserts=False,
        num_devices=num_devices,
    )

    xt = nc.dram_tensor("xt", [KM, P, B], MMDT, kind="ExternalInput").ap()
    xtf = nc.dram_tensor("xtf", [1, B], F32, kind="ExternalInput").ap()
    ytk = nc.dram_tensor("ytk", [KM, P, NLOC], MMDT, kind="ExternalInput").ap()
    ytkf = nc.dram_tensor("ytkf", [1, NLOC], F32, kind="ExternalInput").ap()
    yn = nc.dram_tensor("yn", [NLOC, D], MMDT, kind="ExternalInput").ap()
    s_out = nc.dram_tensor("s", [BT, P, D], F32, kind="ExternalOutput").ap()
    mneg_out = nc.dram_tensor("mneg", [BT, P, 1], F32, kind="ExternalOutput").ap()
    l_out = nc.dram_tensor("l", [BT, P, 1], F32, kind="ExternalOutput").ap()

    with tile.TileContext(nc) as tc:
        with (
            tc.tile_pool(name="const", bufs=1) as const,
            tc.tile_pool(name="dt1", bufs=4) as dt1,
            tc.tile_pool(name="dtf", bufs=1) as dtf_pool,
            tc.tile_pool(name="epool", bufs=2) as epool,
            tc.tile_pool(name="etpool", bufs=2) as etpool,
            tc.tile_pool(name="stat", bufs=4) as stat,
            tc.tile_pool(name="dt2", bufs=8) as dt2,
            tc.tile_pool(name="opool", bufs=4) as opool,
            tc.tile_pool(name="psum", bufs=8, space="PSUM") as psum,
        ):
            # constants: folded/scaled x^T (all k tiles) + identity for transpose
            xt_sb = const.tile([P, KM, B], MMDT)
            nc.sync.dma_start(xt_sb[:], xt.rearrange("k p b -> p k b"))
            xtf_sb = const.tile([P, B], F32)
            nc.vector.memset(xtf_sb[:], 0.0)
            nc.sync.dma_start(xtf_sb[0:1, :], xtf[:])
            ident = const.tile([P, P], F32)
            make_identity(nc, ident)

            # ---- mm1: z'[bt][c] += xt_k.T @ ytk_k  (+ fp32 fold tile last)
            zt = [
                [psum.tile([P, NCH_SZ], F32, tag="ps", name=f"z_{bt}_{c}") for c in range(NCH)]
                for bt in range(BT)
            ]
            KP = 2  # k-tiles per DMA dispatch (faster queue ramp)
            assert KM % KP == 0
            for kp in range(0, KM, KP):
                dpair = dt1.tile([P, KP, NLOC], MMDT, tag="dt", name="dpair")
                nc.sync.dma_start(
                    dpair[:], ytk[kp : kp + KP].rearrange("o p n -> p o n")
                )
                for j in range(KP):
                    k = kp + j
                    for bt in range(BT):
                        lhsT = xt_sb[:, k, bt * P : (bt + 1) * P]
                        for c in range(NCH):
                            nc.tensor.matmul(
                                zt[bt][c][:],
                                lhsT,
                                dpair[:, j, c * NCH_SZ : (c + 1) * NCH_SZ],
                                start=(k == 0),
                                stop=False,
                            )
            ftile = dtf_pool.tile([P, NLOC], F32, tag="dtf", name="dtile_f")
            nc.vector.memset(ftile[:], 0.0)
            nc.sync.dma_start(ftile[0:1, :], ytkf[:])
            for bt in range(BT):
                lhsT = xtf_sb[:, bt * P : (bt + 1) * P]
                for c in range(NCH):
                    nc.tensor.matmul(
                        zt[bt][c][:],
                        lhsT,
                        ftile[:, c * NCH_SZ : (c + 1) * NCH_SZ],
                        start=False,
                        stop=True,
                    )

            # ---- row stats + exp + on-chip transpose of e
            et_tiles = []
            for bt in range(BT):
                mtmp = stat.tile([P, NCH], F32, tag="mtmp")
                for c in range(NCH):
                    nc.vector.reduce_max(mtmp[:, c : c + 1], zt[bt][c][:], axis=AX)
                negm = stat.tile([P, 1], F32, tag="negm")
                nc.vector.reduce_max(negm[:], mtmp[:], axis=AX, negate=True)

                e_sb = epool.tile([P, NLOC], F32, tag="e")
                lpart = stat.tile([P, NCH], F32, tag="lpart")
                for c in range(NCH):
                    nc.scalar.activation(
                        e_sb[:, c * NCH_SZ : (c + 1) * NCH_SZ],
                        zt[bt][c][:],
                        EXP,
                        bias=negm[:],
                        accum_out=lpart[:, c : c + 1],
                    )
                lsum = stat.tile([P, 1], F32, tag="lsum")
                nc.vector.reduce_sum(lsum[:], lpart[:], axis=AX)
                nc.sync.dma_start(mneg_out[bt], negm[:])
                nc.sync.dma_start(l_out[bt], lsum[:])

                et_sb = etpool.tile([P, KT2, P], MMDT, tag="et")
                for kk in range(KT2):
                    ps = psum.tile([P, P], F32, tag="ps")
                    nc.tensor.transpose(
                        ps[:], e_sb[:, kk * P : (kk + 1) * P], ident[:]
                    )
                    nc.any.tensor_copy(out=et_sb[:, kk, :], in_=ps[:])
                et_tiles.append(et_sb)

            # ---- mm2: S[bt] = e[bt] @ Y_loc, done in `passes` column groups
            gstart = [sum(groups[:i]) for i in range(len(groups))]
            for g, group in enumerate(groups):
                d0 = gstart[g] * DCH_SZ
                dw = group * DCH_SZ
                xo = [
                    [psum.tile([P, DCH_SZ], F32, tag="ps", name=f"xo_{g}_{bt}_{c}") for c in range(group)]
                    for bt in range(BT)
                ]
                for k in range(KT2):
                    ytile = dt2.tile([P, dw], MMDT, tag="d2")
                    nc.sync.dma_start(
                        ytile[:], yn[k * P : (k + 1) * P, d0 : d0 + dw]
                    )
                    for bt in range(BT):
                        lhsT = et_tiles[bt][:, k, :]
                        for c in range(group):
                            nc.tensor.matmul(
                                xo[bt][c][:],
                                lhsT,
                                ytile[:, c * DCH_SZ : (c + 1) * DCH_SZ],
                                start=(k == 0),
                                stop=(k == KT2 - 1),
                            )
                for bt in range(BT):
                    for c in range(group):
                        o = opool.tile([P, DCH_SZ], F32, tag="o")
                        nc.any.tensor_copy(out=o[:], in_=xo[bt][c][:])
                        nc.sync.dma_start(
                            s_out[bt][:, d0 + c * DCH_SZ : d0 + (c + 1) * DCH_SZ],
                            o[:],
                        )

    nc.compile()
    return nc


def host_prep(inputs, ts, data_batch, n_cores=8):
    """Schedule coefficients + folded operands, all in the reference's f32."""
    x = np.asarray(inputs, dtype=np.float32)
    y = np.asarray(data_batch, dtype=np.float32)
    B, D = x.shape
    N = y.shape[0]
    NLOC = N // n_cores

    # cosine alpha-bar schedule, matching reference f32 arithmetic
    s = np.float32(0.008)
    tsf = np.asarray(ts).astype(np.float32)
    f = np.cos(((tsf / np.float32(NUM_TIMESTEPS)) + s) / (np.float32(1.0) + s)
               * np.float32(np.pi / 2)) ** 2
    f0 = np.cos(np.float32(s / (1.0 + s) * (np.pi / 2))) ** 2
    alphas = np.clip(f.astype(np.float32) / f0.astype(np.float32), 1e-5, 1.0 - 1e-5)
    alphas = alphas.astype(np.float64)
    v = 1.0 - alphas
    sqrt_a = np.sqrt(alphas)
    g = sqrt_a / v
    h = 0.5 * alphas / v

    # folded stationary operand: [g_b * x_b] tiles + separate [h_b] fold row
    xt_aug = (x.astype(np.float64) * g[:, None]).T.astype(np.float32)
    xt_tiled = np.ascontiguousarray(xt_aug.reshape(D // P, P, B))
    xtf = h.astype(np.float32).reshape(1, B)

    ysq = np.sum(y.astype(np.float64) ** 2, axis=1)

    in_maps = []
    for c in range(n_cores):
        yc = y[c * NLOC : (c + 1) * NLOC]
        ytk = np.ascontiguousarray(yc.T.reshape(D // P, P, NLOC))
        ytkf = (-ysq[c * NLOC : (c + 1) * NLOC]).astype(np.float32).reshape(1, NLOC)
        in_maps.append(
            {
                "xt": xt_tiled,
                "xtf": xtf,
                "ytk": ytk,
                "ytkf": ytkf,
                "yn": np.ascontiguousarray(yc),
            }
        )
    return in_maps, alphas, v, sqrt_a


def host_merge(results, x, alphas, v, sqrt_a):
    """Log-sum-exp merge of the per-core partial softmax results."""
    B, D = x.shape
    m_i = np.stack([-r["mneg"].reshape(B).astype(np.float64) for r in results])
    l_i = np.stack([r["l"].reshape(B).astype(np.float64) for r in results])
    s_i = np.stack([r["s"].reshape(B, D).astype(np.float64) for r in results])

    m = m_i.max(axis=0)
    w = np.exp(m_i - m)                        # [n_cores, B]
    l = (w * l_i).sum(axis=0)                  # [B]
    s = (w[:, :, None] * s_i).sum(axis=0)      # [B, D]
    x0 = s / l[:, None]
    eps = (x.astype(np.float64) - sqrt_a[:, None] * x0) / np.sqrt(v)[:, None]
    return eps.astype(np.float32)


_CACHE = {}


def _get_module():
    if "nc" not in _CACHE:
        _CACHE["nc"] = build_module()
    return _CACHE["nc"]


def kernel(inputs, ts, data_batch, _trace=False):
    n_cores = 8
    x = np.asarray(inputs, dtype=np.float32)
    in_maps, alphas, v, sqrt_a = host_prep(inputs, ts, data_batch, n_cores)
    nc = _get_module()
    res = run_bass_kernel_spmd(nc, in_maps, list(range(n_cores)), trace=_trace)
    out = host_merge(res.results, x, alphas, v, sqrt_a)
    if _trace:
        _CACHE["last_results"] = res
    return out
